# revision 36
# baseline (speedup 1.0000x reference)
"""Trainium2 Bass kernel for the masked cross-frame attention processor.

Contract: kernel(**inputs) takes the FULL unsharded inputs (numpy arrays) and
returns the FULL [8, 1024, 640] float32 output.  Internally the batch axis
(B=8) is data-parallel across 8 NeuronCores; one SPMD Bass program runs on all
cores with per-core input tensors.

Algorithm notes (validated against the reference to ~1e-6 in numpy):
  * nearest-interp of the 256x256 masks to 32x32 is exactly mask[::8, ::8].
  * masked-out KV positions have k == 0, so their score is 0 and they each
    contribute exp(0) == 1 to the softmax denominator and 0 to the numerator.
    We therefore GATHER only the unmasked rows (host-side fancy indexing,
    zero-padded to fixed caps so one compiled NEFF serves all cores) and add
    the constant (2048 - KV_pad) to the denominator.
  * softmax denominators come for free from an extra ones-column at offset 96
    of each head's 97-wide V block (row 96 of the AV psum output is the
    row-sum of P; 96 keeps the DVE read quadrant-aligned).
  * no max-subtraction in softmax: |score * scale| <= ~8 for this data
    distribution (exp is fp32-safe); host fallback covers any pathological
    regeneration of inputs.
"""

import math

import numpy as np

B, S, C = 8, 1024, 640
H = 8
DH = C // H          # 80
DH2 = 97             # per-head V block stride: 80 values, 16 zeros, 1 ones col
VW = H * DH2         # 776
F = 4                # mask/ref frames; batch b uses frame b % F
L1 = 512             # cap for gathered current-frame KV rows (fg mask)
L2 = 640             # cap for gathered reference KV rows (bg mask)
KV = L1 + L2         # 1152 = 9 * 128
NKT = KV // 128      # 9
CORR = float(2 * S - KV)  # dropped/masked kv rows each add exp(0)=1 to denom
SCALE = 1.0 / math.sqrt(DH)
CT = C // 128        # 5 partition tiles of the channel dim

# dtype groups: "f32r" or "bf16" (empirical accuracy/speed tradeoff)
DT_PROJ = "fp16"   # hsT, hsTg, wq, wk, wvi (projection matmul operands)
DT_QK = "fp16"     # qTh, kTh (score matmul operands)
DT_AV = "fp16"     # v_sb, pt (attention-value matmul operands)
DT_Y = "fp16"      # aoP, wop (output projection operands)

_prog_cache = {}


def _build_program():
    """Build (and cache) the SPMD Bass/Tile program."""
    if "nc" in _prog_cache:
        return _prog_cache["nc"]

    from contextlib import ExitStack

    import concourse.bacc as bacc
    import concourse.mybir as mybir
    import concourse.tile as tile

    f32 = mybir.dt.float32
    f32r = mybir.dt.float32r
    u32 = mybir.dt.uint32
    bf16 = mybir.dt.bfloat16
    u16 = mybir.dt.uint16
    f16 = mybir.dt.float16
    dts = {"f32r": f32r, "bf16": bf16, "fp16": f16}
    t_proj, t_qk, t_av, t_y = dts[DT_PROJ], dts[DT_QK], dts[DT_AV], dts[DT_Y]

    def zero_set(ap):
        if ap.dtype in (bf16, f16):
            return nc.gpsimd.memset(ap.bitcast(u16), 0)
        return nc.gpsimd.memset(ap.bitcast(u32), 0)

    def one_set(ap):
        if ap.dtype == bf16:
            return nc.gpsimd.memset(ap.bitcast(u16), 0x3F80)
        if ap.dtype == f16:
            return nc.gpsimd.memset(ap.bitcast(u16), 0x3C00)
        return nc.gpsimd.memset(ap.bitcast(u32), 0x3F800000)


    Exp = mybir.ActivationFunctionType.Exp
    mult = mybir.AluOpType.mult
    add = mybir.AluOpType.add

    nc = bacc.Bacc("TRN2", target_bir_lowering=False, debug=False,
                   enable_asserts=False, num_devices=8)

    # ---- DRAM tensors (per-core views, host-prepared layouts) ----
    d_hsT = nc.dram_tensor("hsT", [C, S], t_proj, kind="ExternalInput").ap()
    d_hsTg = nc.dram_tensor("hsTg", [C, L1], t_proj, kind="ExternalInput").ap()
    d_wq = nc.dram_tensor("wq", [C, C], t_proj, kind="ExternalInput").ap()
    d_wk = nc.dram_tensor("wk", [C, C], t_proj, kind="ExternalInput").ap()
    d_wvi = nc.dram_tensor("wvi", [C, VW], t_proj, kind="ExternalInput").ap()
    d_wop = nc.dram_tensor("wop", [H, 128, C], t_y, kind="ExternalInput").ap()
    d_krth = nc.dram_tensor("krth", [H, DH, L2], t_qk, kind="ExternalInput").ap()
    d_vrg = nc.dram_tensor("vrg", [L2, VW], t_av, kind="ExternalInput").ap()
    d_boc = nc.dram_tensor("boc", [128, CT], f32, kind="ExternalInput").ap()
    d_sel = nc.dram_tensor("sel", [4, 8 * 128], t_y, kind="ExternalInput").ap()
    d_y = nc.dram_tensor("y", [C, S], f32, kind="ExternalOutput").ap()

    def r(ap):
        return ap  # operands are allocated as float32r already

    with tile.TileContext(nc) as tc, ExitStack() as ctx:
        persist = ctx.enter_context(tc.tile_pool(name="persist", bufs=1))

        # ---------- persistent SBUF tensors ----------
        kTh = [persist.tile([128, KV], t_qk, tag=f"kTh{h}", name=f"kTh{h}")
               for h in range(H)]
        qTh = [persist.tile([128, S], t_qk, tag=f"qTh{h}", name=f"qTh{h}")
               for h in range(H)]
        v_sb = [persist.tile([128, VW], t_av, tag=f"v{t}", name=f"v{t}")
                for t in range(NKT)]
        aoP = [persist.tile([128, S], t_y, tag=f"aoP{h}", name=f"aoP{h}")
               for h in range(H)]
        boc = persist.tile([128, CT], f32, tag="boc", name="boc")

        for h in range(H):
            zero_set(aoP[h][64:128, :])

        # ---------- staging pool (lives through attention) ----------
        stg = ctx.enter_context(tc.tile_pool(name="stg", bufs=1))
        drp = ctx.enter_context(tc.tile_pool(name="drp", bufs=1, space="DRAM"))
        sel = stg.tile([4, 8 * 128], t_y, tag="sel", name="sel")
        wo = [stg.tile([128, C], t_y, tag=f"wo{h}", name=f"wo{h}")
              for h in range(H)]
        stage = [stg.tile([1, S], f32, tag=f"stage{h}", name=f"stage{h}")
                 for h in range(H)]
        lrow_dram = drp.tile([H, S], f32, tag="lrow_dram", name="lrow_dram")

        # ---------- PSUM pools: 2 x 2-bank slots + 4 x 1-bank slots --------
        psb = ctx.enter_context(tc.tile_pool(name="psb", bufs=2, space="PSUM"))
        pss = ctx.enter_context(tc.tile_pool(name="pss", bufs=4, space="PSUM"))

        def ps_tile(name):
            return psb.tile([128, S], f32, tag="u", name=name)

        def ps_small(name):
            return pss.tile([128, 512], f32, tag="s", name=name)

        with tc.tile_pool(name="proj", bufs=1) as proj:
            # single wide tiles: one DMA each (Sync-issue latency dominates
            # many small loads)
            hsT_a = proj.tile([128, CT * S], t_proj, tag="hsT", name="hsT")
            hsTg_a = proj.tile([128, CT * L1], t_proj, tag="hsTg", name="hsTg")
            wq_a = proj.tile([128, CT * C], t_proj, tag="wq", name="wq")
            wk_a = proj.tile([128, CT * C], t_proj, tag="wk", name="wk")
            wvi_a = proj.tile([128, CT * VW], t_proj, tag="wvi", name="wvi")
            hsT = [hsT_a[:, k * S:(k + 1) * S] for k in range(CT)]
            hsTg = [hsTg_a[:, k * L1:(k + 1) * L1] for k in range(CT)]
            wq = [wq_a[:, k * C:(k + 1) * C] for k in range(CT)]
            wk = [wk_a[:, k * C:(k + 1) * C] for k in range(CT)]
            wvi = [wvi_a[:, k * VW:(k + 1) * VW] for k in range(CT)]

            def _r(dram):
                return dram.rearrange("(ko p) s -> p ko s", p=128)

            # loads, in consumption order; first k-tile split out so the
            # first matmuls wait only on a small transfer
            for dst_a, dram, w in ((wq_a, d_wq, C), (hsT_a, d_hsT, S),
                                   (hsTg_a, d_hsTg, L1), (wk_a, d_wk, C)):
                nc.sync.dma_start(out=dst_a[:, 0:w], in_=dram[0:128, :])
                nc.sync.dma_start(
                    out=dst_a[:, w:].rearrange("p (ko s) -> p ko s", ko=CT - 1),
                    in_=dram[128:, :].rearrange("(ko p) s -> p ko s", p=128))
            nc.scalar.dma_start(out=wvi_a.rearrange("p (ko s) -> p ko s", ko=CT),
                                in_=_r(d_wvi))
            for h in range(H):
                zero_set(kTh[h][64:128, :])
                zero_set(qTh[h][64:128, :])
                nc.scalar.dma_start(out=kTh[h][0:DH, L1:KV], in_=d_krth[h])
            for t in range(L1 // 128, NKT):  # ref V tiles
                row0 = (t - L1 // 128) * 128
                nc.scalar.dma_start(out=v_sb[t], in_=d_vrg[row0:row0 + 128, :])
            nc.scalar.dma_start(out=sel, in_=d_sel[:])
            for h in range(H):
                nc.scalar.dma_start(out=wo[h], in_=d_wop[h])
            nc.scalar.dma_start(out=boc, in_=d_boc[:])

            def proj_qk_chunk(h, part):
                """part 0/1: qTh[h] halves; part 2: kTh[h] current part."""
                lo, hi = h * DH, (h + 1) * DH
                if part < 2:
                    n = part
                    ps = ps_small(f"qps{h}_{n}")[0:DH, :]
                    for k in range(CT):
                        nc.tensor.matmul(
                            ps, wq[k][:, lo:hi],
                            hsT[k][:, n * 512:(n + 1) * 512],
                            start=(k == 0), stop=(k == CT - 1),
                        )
                    nc.vector.tensor_copy(
                        out=qTh[h][0:DH, n * 512:(n + 1) * 512], in_=ps)
                else:
                    ps = ps_small(f"kps{h}")[0:DH, 0:L1]
                    for k in range(CT):
                        nc.tensor.matmul(ps, wk[k][:, lo:hi], hsTg[k],
                                         start=(k == 0), stop=(k == CT - 1))
                    nc.vector.tensor_copy(out=kTh[h][0:DH, 0:L1], in_=ps)

            def proj_qk(h):
                for part in range(3):
                    proj_qk_chunk(h, part)

            def proj_v(m):
                """current-V tile m (head blocks + ones col)."""
                psa = ps_small(f"vpsA{m}")
                psb2 = ps_small(f"vpsB{m}")[:, 0:VW - 512]
                for k in range(CT):
                    lhsT = hsTg[k][:, m * 128:(m + 1) * 128]
                    nc.tensor.matmul(psa, lhsT, wvi[k][:, 0:512],
                                     start=(k == 0), stop=(k == CT - 1))
                    nc.tensor.matmul(psb2, lhsT, wvi[k][:, 512:VW],
                                     start=(k == 0), stop=(k == CT - 1))
                nc.vector.tensor_copy(out=v_sb[m][:, 0:512], in_=psa)
                nc.vector.tensor_copy(out=v_sb[m][:, 512:VW], in_=psb2)
                for h in range(H):
                    col = h * DH2 + DH2 - 1
                    one_set(v_sb[m][:, col:col + 1])

            def attn_head(h, ptp, filler=None):
                ao = None
                for kt in range(NKT):
                    st = ps_tile(f"st{h}_{kt}")
                    lhsT_k = kTh[h][:, kt * 128:(kt + 1) * 128]
                    for n in range(2):
                        nc.tensor.matmul(
                            st[:, n * 512:(n + 1) * 512], lhsT_k,
                            qTh[h][:, n * 512:(n + 1) * 512],
                            start=True, stop=True,
                        )
                    pt = ptp.tile([128, S], t_av, tag="pt", name="pt")
                    nc.scalar.activation(pt, st, Exp, scale=SCALE)
                    if ao is None:
                        ao = [ps_small(f"ao{h}_{n}")[0:DH2, :]
                              for n in range(2)]
                    lhsT_v = v_sb[kt][:, h * DH2:(h + 1) * DH2]
                    for n in range(2):
                        nc.tensor.matmul(
                            ao[n], lhsT_v,
                            pt[:, n * 512:(n + 1) * 512],
                            start=(kt == 0), stop=(kt == NKT - 1),
                        )
                    if kt == 2 and filler is not None:
                        filler()
                for n in range(2):
                    nc.vector.tensor_scalar_add(
                        stage[h][0:1, n * 512:(n + 1) * 512],
                        ao[n][96:97, :], CORR)
                    nc.vector.tensor_copy(
                        out=aoP[h][0:DH, n * 512:(n + 1) * 512],
                        in_=ao[n][0:DH, :])
                nc.sync.dma_start(out=lrow_dram[h:h + 1, :], in_=stage[h])

            def norm_head(h, rinv):
                for n in range(2):
                    rb = ps_small(f"rb{h}_{n}")
                    nc.tensor.matmul(
                        rb, sel[:, h * 128:(h + 1) * 128],
                        rinv[:, n * 512:(n + 1) * 512],
                        start=True, stop=True,
                    )
                    sl = slice(n * 512, (n + 1) * 512)
                    nc.vector.tensor_tensor(aoP[h][0:DH, sl],
                                            aoP[h][0:DH, sl],
                                            rb[0:DH, :], mult)

            l4a = stg.tile([4, S], f32, tag="l4a", name="l4a")
            rinva_f = stg.tile([4, S], f32, tag="rinva_f", name="rinva_f")
            rinva = stg.tile([4, S], t_y, tag="rinva", name="rinva")
            l4b = stg.tile([4, S], f32, tag="l4b", name="l4b")
            rinvb_f = stg.tile([4, S], f32, tag="rinvb_f", name="rinvb_f")
            rinvb = stg.tile([4, S], t_y, tag="rinvb", name="rinvb")
            l1c = stg.tile([1, S], f32, tag="l1c", name="l1c")
            rinv1_f = stg.tile([1, S], f32, tag="rinv1_f", name="rinv1_f")
            rinv1 = stg.tile([1, S], t_y, tag="rinv1", name="rinv1")
            ones1 = stg.tile([1, 128], t_y, tag="ones1", name="ones1")
            one_set(ones1)
            warm = stg.tile([1, 16], f32, tag="warm", name="warm")
            nc.gpsimd.memset(warm, 0.0)
            nc.scalar.activation(warm, warm, Exp)

            # interleave: projections + early normalization fill ACT slack
            with tc.tile_pool(name="ptp", bufs=8) as ptp:
                proj_qk(0)
                proj_v(0)
                proj_v(1)
                proj_qk(1)
                proj_v(2)
                proj_v(3)
                for h in range(H):
                    if h == 1:
                        # placeholder so the h==7 gather reads finite data
                        nc.sync.dma_start(out=lrow_dram[7:8, :], in_=stage[0])
                    if h + 2 < H:
                        proj_qk(h + 2)
                    elif h == 6:
                        # heads 0..3 denominators ready; normalize them now
                        nc.sync.dma_start(out=l4a, in_=lrow_dram[0:4, :])
                        nc.vector.reciprocal_approx_fast(out=rinva_f, in_=l4a)
                        nc.vector.tensor_copy(out=rinva, in_=rinva_f)
                        for hh in (0, 1):
                            norm_head(hh, rinva)
                    elif h == 7:
                        for hh in (2, 3):
                            norm_head(hh, rinva)
                        # heads 4..6 denominators ready (row 7 placeholder);
                        # only DMA/DVE work here - rb matmuls would block
                        # head 7's attention in the in-order PE queue
                        nc.sync.dma_start(out=l4b, in_=lrow_dram[4:8, :])
                        nc.vector.reciprocal_approx_fast(out=rinvb_f, in_=l4b)
                        nc.vector.tensor_copy(out=rinvb, in_=rinvb_f)
                    attn_head(h, ptp)
                for hh in (4, 5, 6):
                    norm_head(hh, rinvb)

            # ------- normalize heads 4..7 (after attention) -------
            nc.sync.dma_start(out=l4b, in_=lrow_dram[4:8, :])
            nc.vector.reciprocal_approx_fast(out=rinvb_f, in_=l4b)
            nc.vector.tensor_copy(out=rinvb, in_=rinvb_f)
            for h in (4, 5, 6, 7):
                norm_head(h, rinvb)

        # ---------- output projection y = sum_h aoP[h]^T @ Wo_pad[h] ----
        with tc.tile_pool(name="yp", bufs=3) as yp:
                for m in range(S // 128):
                    ps = ps_tile(f"yps{m}")[:, 0:C]
                    for h in range(H):
                        lhsT = aoP[h][:, m * 128:(m + 1) * 128]
                        nc.tensor.matmul(ps[:, 0:512], lhsT, wo[h][:, 0:512],
                                         start=(h == 0), stop=(h == H - 1))
                        nc.tensor.matmul(ps[:, 512:C], lhsT, wo[h][:, 512:C],
                                         start=(h == 0), stop=(h == H - 1))
                    y_sb = yp.tile([128, C], f32, tag="ysb", name="ysb")
                    nc.vector.tensor_tensor(y_sb, boT, ps, add)
                    nc.sync.dma_start(out=d_y[m * 128:(m + 1) * 128, :],
                                      in_=y_sb)

    nc.compile()
    _prog_cache["nc"] = nc
    return nc


def _np_dt(group):
    if group == "bf16":
        import ml_dtypes
        return ml_dtypes.bfloat16
    if group == "fp16":
        return np.float16
    return np.float32


def _prep_inputs(inputs):
    """Host-side sharding: per-core gathered/transposed layouts (numpy only)."""
    tp, tq, ta, ty = (_np_dt(g) for g in (DT_PROJ, DT_QK, DT_AV, DT_Y))
    hs = np.ascontiguousarray(inputs["hidden_states"], dtype=np.float32)
    Wq = np.ascontiguousarray(inputs["Wq"], dtype=np.float32)
    Wk = np.ascontiguousarray(inputs["Wk"], dtype=np.float32)
    Wv = np.ascontiguousarray(inputs["Wv"], dtype=np.float32)
    Wo = np.ascontiguousarray(inputs["Wo"], dtype=np.float32)
    bo = np.ascontiguousarray(inputs["bo"], dtype=np.float32)
    key_ref = np.asarray(inputs["key_ref"], dtype=np.float32)
    value_ref = np.asarray(inputs["value_ref"], dtype=np.float32)
    sm = np.asarray(inputs["source_masks"], dtype=np.float32)
    tm = np.asarray(inputs["target_masks"], dtype=np.float32)

    step = sm.shape[-1] // 32
    frames = []
    overflow = False
    for f in range(F):
        fg = tm[f, 0, ::step, ::step].reshape(S)
        bg = 1.0 - sm[f, 0, ::step, ::step].reshape(S)
        idx1 = np.nonzero(fg)[0]
        idx2 = np.nonzero(bg)[0]
        if len(idx1) > L1 or len(idx2) > L2:
            overflow = True
        frames.append((idx1[:L1], idx2[:L2]))

    Wv_i = np.zeros((C, VW), np.float32)
    for h in range(H):
        Wv_i[:, h * DH2:h * DH2 + DH] = Wv[:, h * DH:(h + 1) * DH]
    Wo_pad = np.zeros((H, 128, C), np.float32)
    for h in range(H):
        Wo_pad[h, 0:DH, :] = Wo[h * DH:(h + 1) * DH, :]
    boc = np.ascontiguousarray(bo.reshape(CT, 128).T)
    sel = np.zeros((4, 8 * 128), np.float32)
    for h in range(H):
        sel[h % 4, h * 128:(h + 1) * 128] = 1.0

    in_maps = []
    for b in range(B):
        idx1, idx2 = frames[b % F]
        n1, n2 = len(idx1), len(idx2)
        hsT = np.ascontiguousarray(hs[b].T)
        hsTg = np.zeros((C, L1), np.float32)
        hsTg[:, :n1] = hs[b].T[:, idx1]
        krth = np.zeros((H, DH, L2), np.float32)
        vrg = np.zeros((L2, VW), np.float32)
        krg = key_ref[b % F][idx2]       # [n2, C]
        vrgath = value_ref[b % F][idx2]  # [n2, C]
        for h in range(H):
            krth[h, :, :n2] = krg[:, h * DH:(h + 1) * DH].T
            vrg[:n2, h * DH2:h * DH2 + DH] = vrgath[:, h * DH:(h + 1) * DH]
            vrg[:, h * DH2 + DH2 - 1] = 1.0
        in_maps.append({
            "hsT": hsT.astype(tp), "hsTg": hsTg.astype(tp),
            "wq": Wq.astype(tp), "wk": Wk.astype(tp), "wvi": Wv_i.astype(tp),
            "wop": Wo_pad.astype(ty),
            "krth": np.ascontiguousarray(krth).astype(tq),
            "vrg": vrg.astype(ta), "boc": boc, "sel": sel.astype(ty),
        })
    return in_maps, overflow


def _host_reference(inputs):
    """Pure-numpy replica of the reference; safety net if gather caps are ever
    exceeded (cannot happen for the spec's input distribution)."""
    hs = np.asarray(inputs["hidden_states"], np.float32)
    Wq, Wk, Wv, Wo = (np.asarray(inputs[k], np.float32)
                      for k in ("Wq", "Wk", "Wv", "Wo"))
    bo = np.asarray(inputs["bo"], np.float32)
    key_ref = np.asarray(inputs["key_ref"], np.float32)
    value_ref = np.asarray(inputs["value_ref"], np.float32)
    sm = np.asarray(inputs["source_masks"], np.float32)
    tm = np.asarray(inputs["target_masks"], np.float32)
    step = sm.shape[-1] // 32
    out = np.zeros((B, S, C), np.float32)
    for b in range(B):
        f = b % F
        fg = tm[f, 0, ::step, ::step].reshape(S, 1)
        bg = 1.0 - sm[f, 0, ::step, ::step].reshape(S, 1)
        q = hs[b] @ Wq
        k = np.concatenate([(hs[b] @ Wk) * fg, key_ref[f] * bg], axis=0)
        v = np.concatenate([(hs[b] @ Wv) * fg, value_ref[f] * bg], axis=0)
        y = np.zeros((S, C), np.float32)
        for h in range(H):
            sl = slice(h * DH, (h + 1) * DH)
            sc = (q[:, sl] @ k[:, sl].T) * SCALE
            sc = sc - sc.max(axis=1, keepdims=True)
            p = np.exp(sc)
            p /= p.sum(axis=1, keepdims=True)
            y[:, sl] = p @ v[:, sl]
        out[b] = y @ Wo + bo
    return out


def kernel(**inputs):
    in_maps, overflow = _prep_inputs(inputs)
    if overflow:
        return _host_reference(inputs)

    from concourse.bass_utils import run_bass_kernel_spmd

    nc = _build_program()
    res = run_bass_kernel_spmd(nc, in_maps, core_ids=list(range(B)))
    out = np.stack([res.results[b]["y"].T for b in range(B)], axis=0)
    return np.ascontiguousarray(out, dtype=np.float32)


# revision 37
# speedup vs baseline: 1.0539x; 1.0539x over previous
"""Trainium2 Bass kernel for the masked cross-frame attention processor.

Contract: kernel(**inputs) takes the FULL unsharded inputs (numpy arrays) and
returns the FULL [8, 1024, 640] float32 output.  Internally the batch axis
(B=8) is data-parallel across 8 NeuronCores; one SPMD Bass program runs on all
cores with per-core input tensors.

Algorithm notes (validated against the reference to ~1e-6 in numpy):
  * nearest-interp of the 256x256 masks to 32x32 is exactly mask[::8, ::8].
  * masked-out KV positions have k == 0, so their score is 0 and they each
    contribute exp(0) == 1 to the softmax denominator and 0 to the numerator.
    We therefore GATHER only the unmasked rows (host-side fancy indexing,
    zero-padded to fixed caps so one compiled NEFF serves all cores) and add
    the constant (2048 - KV_pad) to the denominator.
  * softmax denominators come for free from an extra ones-column at offset 96
    of each head's 97-wide V block (row 96 of the AV psum output is the
    row-sum of P; 96 keeps the DVE read quadrant-aligned).
  * no max-subtraction in softmax: |score * scale| <= ~8 for this data
    distribution (exp is fp32-safe); host fallback covers any pathological
    regeneration of inputs.
"""

import math

import numpy as np

B, S, C = 8, 1024, 640
H = 8
DH = C // H          # 80
DH2 = 97             # per-head V block stride: 80 values, 16 zeros, 1 ones col
VW = H * DH2         # 776
F = 4                # mask/ref frames; batch b uses frame b % F
L1 = 512             # cap for gathered current-frame KV rows (fg mask)
L2 = 640             # cap for gathered reference KV rows (bg mask)
KV = L1 + L2         # 1152 = 9 * 128
NKT = KV // 128      # 9
CORR = float(2 * S - KV)  # dropped/masked kv rows each add exp(0)=1 to denom
SCALE = 1.0 / math.sqrt(DH)
CT = C // 128        # 5 partition tiles of the channel dim

# dtype groups: "f32r" or "bf16" (empirical accuracy/speed tradeoff)
DT_PROJ = "fp16"   # hsT, hsTg, wq, wk, wvi (projection matmul operands)
DT_QK = "fp16"     # qTh, kTh (score matmul operands)
DT_AV = "fp16"     # v_sb, pt (attention-value matmul operands)
DT_Y = "fp16"      # aoP, wop (output projection operands)

_prog_cache = {}


def _build_program():
    """Build (and cache) the SPMD Bass/Tile program."""
    if "nc" in _prog_cache:
        return _prog_cache["nc"]

    from contextlib import ExitStack

    import concourse.bacc as bacc
    import concourse.mybir as mybir
    import concourse.tile as tile

    f32 = mybir.dt.float32
    f32r = mybir.dt.float32r
    u32 = mybir.dt.uint32
    bf16 = mybir.dt.bfloat16
    u16 = mybir.dt.uint16
    f16 = mybir.dt.float16
    dts = {"f32r": f32r, "bf16": bf16, "fp16": f16}
    t_proj, t_qk, t_av, t_y = dts[DT_PROJ], dts[DT_QK], dts[DT_AV], dts[DT_Y]

    def zero_set(ap):
        if ap.dtype in (bf16, f16):
            return nc.gpsimd.memset(ap.bitcast(u16), 0)
        return nc.gpsimd.memset(ap.bitcast(u32), 0)

    def one_set(ap):
        if ap.dtype == bf16:
            return nc.gpsimd.memset(ap.bitcast(u16), 0x3F80)
        if ap.dtype == f16:
            return nc.gpsimd.memset(ap.bitcast(u16), 0x3C00)
        return nc.gpsimd.memset(ap.bitcast(u32), 0x3F800000)


    Exp = mybir.ActivationFunctionType.Exp
    mult = mybir.AluOpType.mult
    add = mybir.AluOpType.add

    nc = bacc.Bacc("TRN2", target_bir_lowering=False, debug=False,
                   enable_asserts=False, num_devices=8)

    # ---- DRAM tensors (per-core views, host-prepared layouts) ----
    d_hsT = nc.dram_tensor("hsT", [C, S], t_proj, kind="ExternalInput").ap()
    d_hsTg = nc.dram_tensor("hsTg", [C, L1], t_proj, kind="ExternalInput").ap()
    d_wq = nc.dram_tensor("wq", [C, C], t_proj, kind="ExternalInput").ap()
    d_wk = nc.dram_tensor("wk", [C, C], t_proj, kind="ExternalInput").ap()
    d_wvi = nc.dram_tensor("wvi", [C, VW], t_proj, kind="ExternalInput").ap()
    d_wop = nc.dram_tensor("wop", [H, 128, C], t_y, kind="ExternalInput").ap()
    d_krth = nc.dram_tensor("krth", [H, DH, L2], t_qk, kind="ExternalInput").ap()
    d_vrg = nc.dram_tensor("vrg", [L2, VW], t_av, kind="ExternalInput").ap()
    d_boc = nc.dram_tensor("boc", [128, CT], f32, kind="ExternalInput").ap()
    d_sel = nc.dram_tensor("sel", [4, 8 * 128], t_y, kind="ExternalInput").ap()
    d_y = nc.dram_tensor("y", [C, S], f32, kind="ExternalOutput").ap()

    def r(ap):
        return ap  # operands are allocated as float32r already

    with tile.TileContext(nc) as tc, ExitStack() as ctx:
        persist = ctx.enter_context(tc.tile_pool(name="persist", bufs=1))

        # ---------- persistent SBUF tensors ----------
        kTh = [persist.tile([128, KV], t_qk, tag=f"kTh{h}", name=f"kTh{h}")
               for h in range(H)]
        qTh = [persist.tile([128, S], t_qk, tag=f"qTh{h}", name=f"qTh{h}")
               for h in range(H)]
        v_sb = [persist.tile([128, VW], t_av, tag=f"v{t}", name=f"v{t}")
                for t in range(NKT)]
        aoP = [persist.tile([128, S], t_y, tag=f"aoP{h}", name=f"aoP{h}")
               for h in range(H)]
        boc = persist.tile([128, CT], f32, tag="boc", name="boc")

        for h in range(H):
            zero_set(aoP[h][64:128, :])

        # ---------- staging pool (lives through attention) ----------
        stg = ctx.enter_context(tc.tile_pool(name="stg", bufs=1))
        drp = ctx.enter_context(tc.tile_pool(name="drp", bufs=1, space="DRAM"))
        sel = stg.tile([4, 8 * 128], t_y, tag="sel", name="sel")
        wo = [stg.tile([128, C], t_y, tag=f"wo{h}", name=f"wo{h}")
              for h in range(H)]
        stage = [stg.tile([1, S], f32, tag=f"stage{h}", name=f"stage{h}")
                 for h in range(H)]
        lrow_dram = drp.tile([H, S], f32, tag="lrow_dram", name="lrow_dram")

        # ---------- PSUM pools: 2 x 2-bank slots + 4 x 1-bank slots --------
        psb = ctx.enter_context(tc.tile_pool(name="psb", bufs=2, space="PSUM"))
        pss = ctx.enter_context(tc.tile_pool(name="pss", bufs=4, space="PSUM"))

        def ps_tile(name):
            return psb.tile([128, S], f32, tag="u", name=name)

        def ps_small(name):
            return pss.tile([128, 512], f32, tag="s", name=name)

        with tc.tile_pool(name="proj", bufs=1) as proj:
            # single wide tiles: one DMA each (Sync-issue latency dominates
            # many small loads)
            hsT_a = proj.tile([128, CT * S], t_proj, tag="hsT", name="hsT")
            hsTg_a = proj.tile([128, CT * L1], t_proj, tag="hsTg", name="hsTg")
            wq_a = proj.tile([128, CT * C], t_proj, tag="wq", name="wq")
            wk_a = proj.tile([128, CT * C], t_proj, tag="wk", name="wk")
            wvi_a = proj.tile([128, CT * VW], t_proj, tag="wvi", name="wvi")
            hsT = [hsT_a[:, k * S:(k + 1) * S] for k in range(CT)]
            hsTg = [hsTg_a[:, k * L1:(k + 1) * L1] for k in range(CT)]
            wq = [wq_a[:, k * C:(k + 1) * C] for k in range(CT)]
            wk = [wk_a[:, k * C:(k + 1) * C] for k in range(CT)]
            wvi = [wvi_a[:, k * VW:(k + 1) * VW] for k in range(CT)]

            def _r(dram):
                return dram.rearrange("(ko p) s -> p ko s", p=128)

            # loads, in consumption order; first k-tile split out so the
            # first matmuls wait only on a small transfer
            for dst_a, dram, w in ((wq_a, d_wq, C), (hsT_a, d_hsT, S),
                                   (hsTg_a, d_hsTg, L1), (wk_a, d_wk, C)):
                nc.sync.dma_start(out=dst_a[:, 0:w], in_=dram[0:128, :])
                nc.sync.dma_start(
                    out=dst_a[:, w:].rearrange("p (ko s) -> p ko s", ko=CT - 1),
                    in_=dram[128:, :].rearrange("(ko p) s -> p ko s", p=128))
            nc.sync.dma_start(out=wvi_a.rearrange("p (ko s) -> p ko s", ko=CT),
                                in_=_r(d_wvi))
            for h in range(H):
                zero_set(kTh[h][64:128, :])
                zero_set(qTh[h][64:128, :])
                nc.sync.dma_start(out=kTh[h][0:DH, L1:KV], in_=d_krth[h])
            for t in range(L1 // 128, NKT):  # ref V tiles
                row0 = (t - L1 // 128) * 128
                nc.sync.dma_start(out=v_sb[t], in_=d_vrg[row0:row0 + 128, :])
            nc.sync.dma_start(out=sel, in_=d_sel[:])
            for h in range(H):
                nc.sync.dma_start(out=wo[h], in_=d_wop[h])
            nc.sync.dma_start(out=boc, in_=d_boc[:])

            def proj_qk_chunk(h, part):
                """part 0/1: qTh[h] halves; part 2: kTh[h] current part."""
                lo, hi = h * DH, (h + 1) * DH
                if part < 2:
                    n = part
                    ps = ps_small(f"qps{h}_{n}")[0:DH, :]
                    for k in range(CT):
                        nc.tensor.matmul(
                            ps, wq[k][:, lo:hi],
                            hsT[k][:, n * 512:(n + 1) * 512],
                            start=(k == 0), stop=(k == CT - 1),
                        )
                    nc.vector.tensor_copy(
                        out=qTh[h][0:DH, n * 512:(n + 1) * 512], in_=ps)
                else:
                    ps = ps_small(f"kps{h}")[0:DH, 0:L1]
                    for k in range(CT):
                        nc.tensor.matmul(ps, wk[k][:, lo:hi], hsTg[k],
                                         start=(k == 0), stop=(k == CT - 1))
                    nc.vector.tensor_copy(out=kTh[h][0:DH, 0:L1], in_=ps)

            def proj_qk(h):
                for part in range(3):
                    proj_qk_chunk(h, part)

            def proj_v(m):
                """current-V tile m (head blocks + ones col)."""
                psa = ps_small(f"vpsA{m}")
                psb2 = ps_small(f"vpsB{m}")[:, 0:VW - 512]
                for k in range(CT):
                    lhsT = hsTg[k][:, m * 128:(m + 1) * 128]
                    nc.tensor.matmul(psa, lhsT, wvi[k][:, 0:512],
                                     start=(k == 0), stop=(k == CT - 1))
                    nc.tensor.matmul(psb2, lhsT, wvi[k][:, 512:VW],
                                     start=(k == 0), stop=(k == CT - 1))
                nc.vector.tensor_copy(out=v_sb[m][:, 0:512], in_=psa)
                nc.vector.tensor_copy(out=v_sb[m][:, 512:VW], in_=psb2)
                for h in range(H):
                    col = h * DH2 + DH2 - 1
                    one_set(v_sb[m][:, col:col + 1])

            def attn_head(h, ptp, filler=None):
                ao = None
                for kt in range(NKT):
                    st = ps_tile(f"st{h}_{kt}")
                    lhsT_k = kTh[h][:, kt * 128:(kt + 1) * 128]
                    for n in range(2):
                        nc.tensor.matmul(
                            st[:, n * 512:(n + 1) * 512], lhsT_k,
                            qTh[h][:, n * 512:(n + 1) * 512],
                            start=True, stop=True,
                        )
                    pt = ptp.tile([128, S], t_av, tag="pt", name="pt")
                    nc.scalar.activation(pt, st, Exp, scale=SCALE)
                    if ao is None:
                        ao = [ps_small(f"ao{h}_{n}")[0:DH2, :]
                              for n in range(2)]
                    lhsT_v = v_sb[kt][:, h * DH2:(h + 1) * DH2]
                    for n in range(2):
                        nc.tensor.matmul(
                            ao[n], lhsT_v,
                            pt[:, n * 512:(n + 1) * 512],
                            start=(kt == 0), stop=(kt == NKT - 1),
                        )
                    if kt == 2 and filler is not None:
                        filler()
                for n in range(2):
                    nc.vector.tensor_scalar_add(
                        stage[h][0:1, n * 512:(n + 1) * 512],
                        ao[n][96:97, :], CORR)
                    nc.vector.tensor_copy(
                        out=aoP[h][0:DH, n * 512:(n + 1) * 512],
                        in_=ao[n][0:DH, :])
                nc.sync.dma_start(out=lrow_dram[h:h + 1, :], in_=stage[h])

            def norm_head(h, rinv):
                for n in range(2):
                    rb = ps_small(f"rb{h}_{n}")
                    nc.tensor.matmul(
                        rb, sel[:, h * 128:(h + 1) * 128],
                        rinv[:, n * 512:(n + 1) * 512],
                        start=True, stop=True,
                    )
                    sl = slice(n * 512, (n + 1) * 512)
                    nc.vector.tensor_tensor(aoP[h][0:DH, sl],
                                            aoP[h][0:DH, sl],
                                            rb[0:DH, :], mult)

            l4a = stg.tile([4, S], f32, tag="l4a", name="l4a")
            rinva_f = stg.tile([4, S], f32, tag="rinva_f", name="rinva_f")
            rinva = stg.tile([4, S], t_y, tag="rinva", name="rinva")
            l4b = stg.tile([4, S], f32, tag="l4b", name="l4b")
            rinvb_f = stg.tile([4, S], f32, tag="rinvb_f", name="rinvb_f")
            rinvb = stg.tile([4, S], t_y, tag="rinvb", name="rinvb")
            l1c = stg.tile([1, S], f32, tag="l1c", name="l1c")
            rinv1_f = stg.tile([1, S], f32, tag="rinv1_f", name="rinv1_f")
            rinv1 = stg.tile([1, S], t_y, tag="rinv1", name="rinv1")
            ones1 = stg.tile([1, 128], t_y, tag="ones1", name="ones1")
            one_set(ones1)
            warm = stg.tile([1, 16], f32, tag="warm", name="warm")
            nc.gpsimd.memset(warm, 0.0)
            nc.scalar.activation(warm, warm, Exp)

            # interleave: projections + early normalization fill ACT slack
            with tc.tile_pool(name="ptp", bufs=8) as ptp:
                proj_qk(0)
                proj_v(0)
                proj_v(1)
                proj_qk(1)
                proj_v(2)
                proj_v(3)
                for h in range(H):
                    if h == 1:
                        # placeholder so the h==7 gather reads finite data
                        nc.sync.dma_start(out=lrow_dram[7:8, :], in_=stage[0])
                    if h + 2 < H:
                        proj_qk(h + 2)
                    elif h == 6:
                        # heads 0..3 denominators ready; normalize them now
                        nc.sync.dma_start(out=l4a, in_=lrow_dram[0:4, :])
                        nc.vector.reciprocal_approx_fast(out=rinva_f, in_=l4a)
                        nc.vector.tensor_copy(out=rinva, in_=rinva_f)
                        for hh in (0, 1):
                            norm_head(hh, rinva)
                    elif h == 7:
                        for hh in (2, 3):
                            norm_head(hh, rinva)
                        # heads 4..6 denominators ready (row 7 placeholder);
                        # only DMA/DVE work here - rb matmuls would block
                        # head 7's attention in the in-order PE queue
                        nc.sync.dma_start(out=l4b, in_=lrow_dram[4:8, :])
                        nc.vector.reciprocal_approx_fast(out=rinvb_f, in_=l4b)
                        nc.vector.tensor_copy(out=rinvb, in_=rinvb_f)
                    attn_head(h, ptp)
                for hh in (4, 5, 6):
                    norm_head(hh, rinvb)

            # ------- normalize heads 4..7 (after attention) -------
            nc.sync.dma_start(out=l4b, in_=lrow_dram[4:8, :])
            nc.vector.reciprocal_approx_fast(out=rinvb_f, in_=l4b)
            nc.vector.tensor_copy(out=rinvb, in_=rinvb_f)
            for h in (4, 5, 6, 7):
                norm_head(h, rinvb)

        # ---------- output projection y = sum_h aoP[h]^T @ Wo_pad[h] ----
        with tc.tile_pool(name="yp", bufs=3) as yp:
                for m in range(S // 128):
                    ps = ps_tile(f"yps{m}")[:, 0:C]
                    for h in range(H):
                        lhsT = aoP[h][:, m * 128:(m + 1) * 128]
                        nc.tensor.matmul(ps[:, 0:512], lhsT, wo[h][:, 0:512],
                                         start=(h == 0), stop=(h == H - 1))
                        nc.tensor.matmul(ps[:, 512:C], lhsT, wo[h][:, 512:C],
                                         start=(h == 0), stop=(h == H - 1))
                    y_sb = yp.tile([128, C], f32, tag="ysb", name="ysb")
                    nc.vector.tensor_tensor(y_sb, boT, ps, add)
                    nc.sync.dma_start(out=d_y[m * 128:(m + 1) * 128, :],
                                      in_=y_sb)

    nc.compile()
    _prog_cache["nc"] = nc
    return nc


def _np_dt(group):
    if group == "bf16":
        import ml_dtypes
        return ml_dtypes.bfloat16
    if group == "fp16":
        return np.float16
    return np.float32


def _prep_inputs(inputs):
    """Host-side sharding: per-core gathered/transposed layouts (numpy only)."""
    tp, tq, ta, ty = (_np_dt(g) for g in (DT_PROJ, DT_QK, DT_AV, DT_Y))
    hs = np.ascontiguousarray(inputs["hidden_states"], dtype=np.float32)
    Wq = np.ascontiguousarray(inputs["Wq"], dtype=np.float32)
    Wk = np.ascontiguousarray(inputs["Wk"], dtype=np.float32)
    Wv = np.ascontiguousarray(inputs["Wv"], dtype=np.float32)
    Wo = np.ascontiguousarray(inputs["Wo"], dtype=np.float32)
    bo = np.ascontiguousarray(inputs["bo"], dtype=np.float32)
    key_ref = np.asarray(inputs["key_ref"], dtype=np.float32)
    value_ref = np.asarray(inputs["value_ref"], dtype=np.float32)
    sm = np.asarray(inputs["source_masks"], dtype=np.float32)
    tm = np.asarray(inputs["target_masks"], dtype=np.float32)

    step = sm.shape[-1] // 32
    frames = []
    overflow = False
    for f in range(F):
        fg = tm[f, 0, ::step, ::step].reshape(S)
        bg = 1.0 - sm[f, 0, ::step, ::step].reshape(S)
        idx1 = np.nonzero(fg)[0]
        idx2 = np.nonzero(bg)[0]
        if len(idx1) > L1 or len(idx2) > L2:
            overflow = True
        frames.append((idx1[:L1], idx2[:L2]))

    Wv_i = np.zeros((C, VW), np.float32)
    for h in range(H):
        Wv_i[:, h * DH2:h * DH2 + DH] = Wv[:, h * DH:(h + 1) * DH]
    Wo_pad = np.zeros((H, 128, C), np.float32)
    for h in range(H):
        Wo_pad[h, 0:DH, :] = Wo[h * DH:(h + 1) * DH, :]
    boc = np.ascontiguousarray(bo.reshape(CT, 128).T)
    sel = np.zeros((4, 8 * 128), np.float32)
    for h in range(H):
        sel[h % 4, h * 128:(h + 1) * 128] = 1.0

    in_maps = []
    for b in range(B):
        idx1, idx2 = frames[b % F]
        n1, n2 = len(idx1), len(idx2)
        hsT = np.ascontiguousarray(hs[b].T)
        hsTg = np.zeros((C, L1), np.float32)
        hsTg[:, :n1] = hs[b].T[:, idx1]
        krth = np.zeros((H, DH, L2), np.float32)
        vrg = np.zeros((L2, VW), np.float32)
        krg = key_ref[b % F][idx2]       # [n2, C]
        vrgath = value_ref[b % F][idx2]  # [n2, C]
        for h in range(H):
            krth[h, :, :n2] = krg[:, h * DH:(h + 1) * DH].T
            vrg[:n2, h * DH2:h * DH2 + DH] = vrgath[:, h * DH:(h + 1) * DH]
            vrg[:, h * DH2 + DH2 - 1] = 1.0
        in_maps.append({
            "hsT": hsT.astype(tp), "hsTg": hsTg.astype(tp),
            "wq": Wq.astype(tp), "wk": Wk.astype(tp), "wvi": Wv_i.astype(tp),
            "wop": Wo_pad.astype(ty),
            "krth": np.ascontiguousarray(krth).astype(tq),
            "vrg": vrg.astype(ta), "boc": boc, "sel": sel.astype(ty),
        })
    return in_maps, overflow


def _host_reference(inputs):
    """Pure-numpy replica of the reference; safety net if gather caps are ever
    exceeded (cannot happen for the spec's input distribution)."""
    hs = np.asarray(inputs["hidden_states"], np.float32)
    Wq, Wk, Wv, Wo = (np.asarray(inputs[k], np.float32)
                      for k in ("Wq", "Wk", "Wv", "Wo"))
    bo = np.asarray(inputs["bo"], np.float32)
    key_ref = np.asarray(inputs["key_ref"], np.float32)
    value_ref = np.asarray(inputs["value_ref"], np.float32)
    sm = np.asarray(inputs["source_masks"], np.float32)
    tm = np.asarray(inputs["target_masks"], np.float32)
    step = sm.shape[-1] // 32
    out = np.zeros((B, S, C), np.float32)
    for b in range(B):
        f = b % F
        fg = tm[f, 0, ::step, ::step].reshape(S, 1)
        bg = 1.0 - sm[f, 0, ::step, ::step].reshape(S, 1)
        q = hs[b] @ Wq
        k = np.concatenate([(hs[b] @ Wk) * fg, key_ref[f] * bg], axis=0)
        v = np.concatenate([(hs[b] @ Wv) * fg, value_ref[f] * bg], axis=0)
        y = np.zeros((S, C), np.float32)
        for h in range(H):
            sl = slice(h * DH, (h + 1) * DH)
            sc = (q[:, sl] @ k[:, sl].T) * SCALE
            sc = sc - sc.max(axis=1, keepdims=True)
            p = np.exp(sc)
            p /= p.sum(axis=1, keepdims=True)
            y[:, sl] = p @ v[:, sl]
        out[b] = y @ Wo + bo
    return out


def kernel(**inputs):
    in_maps, overflow = _prep_inputs(inputs)
    if overflow:
        return _host_reference(inputs)

    from concourse.bass_utils import run_bass_kernel_spmd

    nc = _build_program()
    res = run_bass_kernel_spmd(nc, in_maps, core_ids=list(range(B)))
    out = np.stack([res.results[b]["y"].T for b in range(B)], axis=0)
    return np.ascontiguousarray(out, dtype=np.float32)
